# revision 1
# baseline (speedup 1.0000x reference)
"""Trainium2 Bass kernel for single-head causal attention.

Problem: x[B=4,T=2048,C=1024] -> q,k,v = x@Wq/Wk/Wv [T,64] -> causal softmax(q k^T/sqrt(C)) @ v.

Sharding: 8 cores = 4 batches x 2 query-halves (sequence-parallel queries,
replicated weights). Each core computes K/V projections for the full
sequence and attention for its 1024 queries.

SPMD-uniform trick: the time axis of each core's x^T copy is permuted so the
core's OWN query half comes first (columns 0-1023), the other half after.
Then the causal block structure is identical on every core:
  - k-tiles j=0..7  (own half): lower-triangular blocks, diagonal gets a
    constant 128x128 triangular mask; blocks above the diagonal are skipped.
  - k-tiles j=8..15 (other half): full rectangle whose validity differs only
    by DATA: an exp-bias "gate" per core (0.0 => keep, -60 => exp ~ 0).
Softmax normalization is fused into the AV matmul by appending a ones column
to V (output row 64 = sum of exp); division happens host-side on gather.

Device layout: S^T = k_tile^T-stationary x q^T-moving so the softmax free
dim is q and P^T feeds AV directly with V-natural stationary (V transposed
on-device via the DMA xbar, bf16).
"""

import numpy as np
import ml_dtypes

B, T, C, H = 4, 2048, 1024, 64
TQ = 1024          # queries per core
NT = 2048          # kv length per core
NCH = C // 128     # 8 contraction chunks
NKT = NT // 128    # 16 k-tiles
SCALE = 1.0 / 32.0  # 1/sqrt(C)
VSTRIDE = 80       # bf16 cols per v' tile slot (64 v + 1 ones + pad, 32B-aligned)

_prog_cache = {}


def _build_program():
    import concourse.mybir as mybir
    from concourse import bacc
    from concourse.tile import TileContext

    fp32 = mybir.dt.float32
    bf16 = mybir.dt.bfloat16
    Exp = mybir.ActivationFunctionType.Exp

    nc = bacc.Bacc("TRN2", target_bir_lowering=False, debug=False)

    xt_d = nc.dram_tensor("xt", [C, NT], bf16, kind="ExternalInput")
    wqk_d = nc.dram_tensor("wqk", [C, 128], bf16, kind="ExternalInput")
    wv_d = nc.dram_tensor("wv", [C, H], bf16, kind="ExternalInput")
    gate_d = nc.dram_tensor("gate", [128, 1], fp32, kind="ExternalInput")
    tri_d = nc.dram_tensor("tri", [128, 128], bf16, kind="ExternalInput")
    idn_d = nc.dram_tensor("idn", [64, 64], bf16, kind="ExternalInput")
    out_d = nc.dram_tensor("outT", [H + 1, TQ], fp32, kind="ExternalOutput")

    with TileContext(nc) as tc:
        with (
            tc.tile_pool(name="xtp", bufs=1) as xt_pool,
            tc.tile_pool(name="cst", bufs=1) as cst,
            tc.tile_pool(name="prj", bufs=1) as prj,
            tc.tile_pool(name="ptp", bufs=8) as ptp,
            tc.tile_pool(name="psA", bufs=1, space="PSUM") as psA,
            tc.tile_pool(name="psB", bufs=1, space="PSUM") as psB,
            tc.tile_pool(name="psS", bufs=2, space="PSUM") as psS,
            tc.tile_pool(name="psO", bufs=2, space="PSUM") as psO,
        ):
            # constants / weights
            wqk_sb = cst.tile([128, NCH, 128], bf16, tag="wqk")
            nc.sync.dma_start(out=wqk_sb[:], in_=wqk_d.rearrange("(o p) m -> p o m", p=128))
            wv_sb = cst.tile([128, NCH, H], bf16, tag="wv")
            nc.sync.dma_start(out=wv_sb[:], in_=wv_d.rearrange("(o p) m -> p o m", p=128))
            gate_sb = cst.tile([128, 1], fp32, tag="gate")
            nc.sync.dma_start(out=gate_sb[:], in_=gate_d[:])
            tri_sb = cst.tile([128, 128], bf16, tag="tri")
            nc.sync.dma_start(out=tri_sb[:], in_=tri_d[:])
            idn_sb = cst.tile([64, 64], bf16, tag="idn")
            nc.sync.dma_start(out=idn_sb[:], in_=idn_d[:])

            # x^T chunks (C on partitions)
            xt_sb = []
            for c in range(NCH):
                t = xt_pool.tile([128, NT], bf16, tag=f"xt{c}")
                nc.sync.dma_start(out=t[:], in_=xt_d[c * 128:(c + 1) * 128, :])
                xt_sb.append(t)

            # persistent projection outputs
            qT_sb = prj.tile([64, TQ], fp32, tag="qT")
            kT_sb = prj.tile([64, NT], fp32, tag="kT")
            vT_sb = prj.tile([64, NT], bf16, tag="vT")
            vp_sb = prj.tile([128, NKT * VSTRIDE], bf16, tag="vp")
            o_sb = prj.tile([H + 1, TQ], fp32, tag="osb")

            # ones column (col 64 of each VSTRIDE block) for the l-row trick
            nc.vector.memset(
                vp_sb.rearrange("p (t c) -> p t c", c=VSTRIDE)[:, :, 64:65], 1.0
            )

            # PE warmup during the initial x^T DMA (copied to a dummy spot
            # so the verifier sees a reader)
            wq_flat = wqk_sb.rearrange("p o m -> p (o m)")
            scratch = psS.tile([128, 512], fp32, tag="s")
            for _ in range(8):
                nc.tensor.matmul(
                    scratch[:], wqk_sb[:, 0, :], wq_flat[:, 0:512],
                    start=True, stop=True,
                )
            nc.vector.tensor_copy(out=vp_sb[:, 0:64], in_=scratch[:, 0:64])

            # AV accumulators (allocated later, after the v transposes
            # borrow the psO slots)
            o_ps = []

            def emit_quarter(tq):
                """Project 512 time columns: q^T/k^T (packed) and v^T."""
                sl = slice(tq * 512, (tq + 1) * 512)
                qk_ps = psA.tile([128, 512], fp32, tag="qk")
                v_ps = psB.tile([64, 512], fp32, tag="pv")
                for c in range(NCH):
                    nc.tensor.matmul(
                        qk_ps[:], wqk_sb[:, c, :], xt_sb[c][:, sl],
                        start=(c == 0), stop=(c == NCH - 1),
                    )
                for c in range(NCH):
                    nc.tensor.matmul(
                        v_ps[:], wv_sb[:, c, :], xt_sb[c][:, sl],
                        start=(c == 0), stop=(c == NCH - 1),
                    )
                if tq < TQ // 512:
                    nc.vector.tensor_copy(out=qT_sb[:, sl], in_=qk_ps[0:64, :])
                nc.vector.tensor_copy(out=kT_sb[:, sl], in_=qk_ps[64:128, :])
                nc.vector.tensor_copy(out=vT_sb[:, sl], in_=v_ps[:])

            pt_tiles = {}

            def emit_S(j):
                """One k-tile: S^T matmul, exp (with gate bias), diagonal mask."""
                a0 = 128 * j if j < 8 else 0
                s_ps = psS.tile([128, 1024], fp32, tag="s")
                for b in (0, 1):
                    lo, hi = max(a0, 512 * b), 512 * (b + 1)
                    if lo < hi:
                        nc.tensor.matmul(
                            s_ps[:, lo:hi],
                            kT_sb[:, 128 * j: 128 * (j + 1)],
                            qT_sb[:, lo:hi],
                            start=True, stop=True,
                        )
                pt = ptp.tile([128, 1024], bf16, tag="pt")
                bias = gate_sb[:, 0:1] if j >= 8 else 0.0
                nc.scalar.activation(
                    pt[:, a0:1024], s_ps[:, a0:1024], Exp, bias=bias, scale=SCALE
                )
                if j < 8:
                    nc.vector.tensor_mul(
                        pt[:, 128 * j: 128 * (j + 1)],
                        pt[:, 128 * j: 128 * (j + 1)],
                        tri_sb[:],
                    )
                pt_tiles[j] = pt

            def emit_AV(j):
                a0 = 128 * j if j < 8 else 0
                pt = pt_tiles.pop(j)
                for b in (0, 1):
                    lo, hi = max(a0, 512 * b), 512 * (b + 1)
                    if lo < hi:
                        nc.tensor.matmul(
                            o_ps[b][:, lo - 512 * b: hi - 512 * b],
                            vp_sb[:, VSTRIDE * j: VSTRIDE * j + 65],
                            pt[:, lo:hi],
                            start=(j == 0), stop=(j == NKT - 1),
                            skip_group_check=True,
                        )

            emit_quarter(0)
            emit_quarter(1)
            emit_S(0)
            emit_S(1)
            emit_quarter(2)
            emit_S(2)
            emit_S(3)
            emit_quarter(3)
            emit_S(4)
            emit_S(5)

            # v^T -> v-natural via DMA xbar transpose (bf16)
            for t in range(NKT):
                nc.sync.dma_start(
                    out=vp_sb[:, VSTRIDE * t: VSTRIDE * t + 64],
                    in_=vT_sb[:, 128 * t: 128 * (t + 1)],
                    transpose=True,
                )

            o_ps0 = psO.tile([H + 1, 512], fp32, tag="o")
            o_ps1 = psO.tile([H + 1, 512], fp32, tag="o")
            o_ps.extend([o_ps0, o_ps1])

            for j in range(6):
                emit_AV(j)
            for j in range(6, NKT):
                emit_S(j)
                emit_AV(j)

            for b in (0, 1):
                nc.vector.tensor_copy(
                    out=o_sb[:, 512 * b: 512 * (b + 1)], in_=o_ps[b][:]
                )
            nc.sync.dma_start(out=out_d[:], in_=o_sb[:])

    nc.finalize()
    return nc


def _get_program():
    if "nc" not in _prog_cache:
        _prog_cache["nc"] = _build_program()
    return _prog_cache["nc"]


def make_in_maps(x, Wq, Wk, Wv):
    bf16 = ml_dtypes.bfloat16
    wqk = np.concatenate([Wq, Wk], axis=1).astype(bf16)  # [C, 128]
    wv = np.ascontiguousarray(Wv.astype(bf16))
    tri = np.triu(np.ones((128, 128), np.float32)).astype(bf16)  # tri[k,q]=1 iff q>=k
    idn = np.eye(64, dtype=np.float32).astype(bf16)
    in_maps = []
    for core in range(8):
        b, r = core // 2, core % 2
        qs = r * TQ
        other = (1 - r) * TQ
        xb = np.asarray(x[b])
        xt = np.concatenate([xb[qs:qs + TQ], xb[other:other + TQ]], axis=0).T
        gate = np.full((128, 1), 0.0 if r == 1 else -60.0, np.float32)
        in_maps.append({
            "xt": np.ascontiguousarray(xt).astype(bf16),
            "wqk": wqk,
            "wv": wv,
            "gate": gate,
            "tri": tri,
            "idn": idn,
        })
    return in_maps


def postprocess(results):
    out = np.empty((B, T, H), np.float32)
    for core in range(8):
        b, r = core // 2, core % 2
        qs = r * TQ
        oT = results[core]["outT"]  # [65, 1024]
        out[b, qs:qs + TQ] = (oT[:H] / oT[H:H + 1]).T
    return out


def kernel(x, mask, Wq, Wk, Wv, _trace=False, _tracedir=None):
    from concourse import bass_utils

    nc = _get_program()
    in_maps = make_in_maps(np.asarray(x, np.float32), np.asarray(Wq, np.float32),
                           np.asarray(Wk, np.float32), np.asarray(Wv, np.float32))
    res = bass_utils.run_bass_kernel_spmd(
        nc, in_maps, core_ids=list(range(8)),
        trace=_trace, tmpdir=_tracedir,
    )
    out = postprocess(res.results)
    if _trace:
        return out, res
    return out



# revision 3
# speedup vs baseline: 1.2231x; 1.2231x over previous
"""Trainium2 Bass kernel for single-head causal attention.

Problem: x[B=4,T=2048,C=1024] -> q,k,v = x@Wq/Wk/Wv [T,64] -> causal softmax(q k^T/sqrt(C)) @ v.

Sharding: 8 cores = 4 batches x 2 interleaved query-tile sets. Core r of a
batch owns global 128-row q-tiles {2m+r : m=0..7} -- interleaving balances
the causal triangle exactly (68 vs 68 blocks) instead of 36 vs 100 for
contiguous halves.

SPMD-uniform trick: each core's x^T copy is column-permuted so its OWN
q-tiles come first (local tiles 0-7), the peer's after (local 8-15). The
causal block structure is then core-independent:
  - local k-tile p<8  (own tiles, global 2p+r): q-window m >= p; the m==p
    block is the diagonal -> multiply by a constant 128x128 triangle.
  - local k-tile p>=8 (peer tiles, global 2(p-8)+1-r): q-window m >= p-8;
    the boundary block m==p-8 is valid only for r==1, gated by a per-core
    exp bias (0 keeps, -60 zeroes). All other blocks are fully valid.
Softmax normalization is fused into the AV matmul by appending a ones
column to V (output row 64 = sum of exp); division happens host-side.

Device layout: S^T = k_tile-stationary x q^T-moving in bf16 (4x faster
than fp32 on the PE) so the softmax free dim is q and P^T feeds AV with
V-natural stationary. V is transposed on the PE (identity-moving
transpose) instead of DMA-xbar. x^T is DMA'd in halves (own first) per
128-row contraction chunk, spread across the sync/scalar/gpsimd DMA
queues, so projection matmuls start as soon as the first chunk lands.
"""

import numpy as np
import ml_dtypes

B, T, C, H = 4, 2048, 1024, 64
TQ = 1024          # queries per core
NT = 2048          # kv length per core
NCH = C // 128     # 8 contraction chunks
NKT = NT // 128    # 16 local k-tiles
SCALE = 1.0 / 32.0  # 1/sqrt(C)
VSTRIDE = 80       # bf16 cols per v' tile slot (64 v + 1 ones + pad)

_prog_cache = {}


def _build_program():
    import concourse.mybir as mybir
    from concourse import bacc
    from concourse.tile import TileContext

    fp32 = mybir.dt.float32
    bf16 = mybir.dt.bfloat16
    Exp = mybir.ActivationFunctionType.Exp

    nc = bacc.Bacc("TRN2", target_bir_lowering=False, debug=False)

    xt_d = nc.dram_tensor("xt", [C, NT], bf16, kind="ExternalInput")
    wqk_d = nc.dram_tensor("wqk", [C, 128], bf16, kind="ExternalInput")
    wv_d = nc.dram_tensor("wv", [C, H], bf16, kind="ExternalInput")
    gate_d = nc.dram_tensor("gate", [128, 1], fp32, kind="ExternalInput")
    tri_d = nc.dram_tensor("tri", [128, 128], bf16, kind="ExternalInput")
    idn_d = nc.dram_tensor("idn", [64, 64], bf16, kind="ExternalInput")
    out_d = nc.dram_tensor("outT", [H + 1, TQ], fp32, kind="ExternalOutput")

    with TileContext(nc) as tc:
        with (
            tc.tile_pool(name="xtp", bufs=1) as xt_pool,
            tc.tile_pool(name="cst", bufs=1) as cst,
            tc.tile_pool(name="prj", bufs=1) as prj,
            tc.tile_pool(name="ptp", bufs=4) as ptp,
            tc.tile_pool(name="psA", bufs=2, space="PSUM") as psA,
            tc.tile_pool(name="psB", bufs=1, space="PSUM") as psB,
            tc.tile_pool(name="psS", bufs=3, space="PSUM") as psS,
            tc.tile_pool(name="psO", bufs=1, space="PSUM") as psO,
        ):
            # constants / weights (sync queue, issued first)
            wqk_sb = cst.tile([128, NCH, 128], bf16, tag="wqk")
            nc.sync.dma_start(out=wqk_sb[:], in_=wqk_d.rearrange("(o p) m -> p o m", p=128))
            wv_sb = cst.tile([128, NCH, H], bf16, tag="wv")
            nc.sync.dma_start(out=wv_sb[:], in_=wv_d.rearrange("(o p) m -> p o m", p=128))
            gate_sb = cst.tile([128, 1], fp32, tag="gate")
            nc.sync.dma_start(out=gate_sb[:], in_=gate_d[:])
            tri_sb = cst.tile([128, 128], bf16, tag="tri")
            nc.sync.dma_start(out=tri_sb[:], in_=tri_d[:])
            idn_sb = cst.tile([64, 64], bf16, tag="idn")
            nc.sync.dma_start(out=idn_sb[:], in_=idn_d[:])

            # x^T chunks (C on partitions); own half (cols 0:1024) first,
            # spread across three DMA queues
            xt_sb = [
                xt_pool.tile([128, NT], bf16, tag=f"xt{c}", name=f"xt{c}")
                for c in range(NCH)
            ]
            for c in range(NCH):
                eng = nc.sync if c % 2 == 0 else nc.scalar
                eng.dma_start(out=xt_sb[c][:, 0:TQ], in_=xt_d[c * 128:(c + 1) * 128, 0:TQ])
            for c in range(NCH):
                nc.gpsimd.dma_start(out=xt_sb[c][:, TQ:NT], in_=xt_d[c * 128:(c + 1) * 128, TQ:NT])

            # persistent projection outputs (all bf16)
            qT_sb = prj.tile([64, TQ], bf16, tag="qT")
            kT_sb = prj.tile([64, NT], bf16, tag="kT")
            vT_sb = prj.tile([64, NT], bf16, tag="vT")
            vp_sb = prj.tile([128, NKT, VSTRIDE], bf16, tag="vp")
            o_sb = prj.tile([H + 1, TQ], fp32, tag="osb")
            warm_sb = prj.tile([128, 64], bf16, tag="warm")

            # ones column (col 64 of each VSTRIDE block) for the l-row trick
            nc.vector.memset(vp_sb[:, :, 64:65], 1.0)

            # PE p-state warmup during the initial x^T DMA (result copied
            # out so the verifier sees a reader)
            wq_flat = wqk_sb.rearrange("p o m -> p (o m)")
            warm_ps = psS.tile([128, 512], fp32, tag="s")
            for _ in range(6):
                nc.tensor.matmul(
                    warm_ps[:], wqk_sb[:, 0, :], wq_flat[:, 0:512],
                    start=True, stop=True,
                )
            nc.vector.tensor_copy(out=warm_sb[:, 0:64], in_=warm_ps[:, 0:64])
            # warm the Exp activation table early (1.3us load off the
            # critical path)
            nc.scalar.activation(warm_sb[0:128, 0:1], warm_ps[:, 0:1], Exp, scale=SCALE)

            def emit_quarter(tq):
                """Project 512 time columns: q^T/k^T (packed) and v^T."""
                sl = slice(tq * 512, (tq + 1) * 512)
                qk_ps = psA.tile([128, 512], fp32, tag="qk")
                for c in range(NCH):
                    nc.tensor.matmul(
                        qk_ps[:], wqk_sb[:, c, :], xt_sb[c][:, sl],
                        start=(c == 0), stop=(c == NCH - 1),
                    )
                v_ps = psB.tile([64, 512], fp32, tag="pv")
                for c in range(NCH):
                    nc.tensor.matmul(
                        v_ps[:], wv_sb[:, c, :], xt_sb[c][:, sl],
                        start=(c == 0), stop=(c == NCH - 1),
                    )
                if tq < TQ // 512:
                    nc.vector.tensor_copy(out=qT_sb[:, sl], in_=qk_ps[0:64, :])
                nc.vector.tensor_copy(out=kT_sb[:, sl], in_=qk_ps[64:128, :])
                nc.vector.tensor_copy(out=vT_sb[:, sl], in_=v_ps[:])

            def emit_vtrans(tq):
                """PE-transpose v^T tiles 4*tq..4*tq+3 into vp (v-natural)."""
                vt_ps = psS.tile([128, 8, 64], bf16, tag="s")
                for i in range(4):
                    t = 4 * tq + i
                    nc.tensor.transpose(
                        vt_ps[:, i, :], vT_sb[:, 128 * t:128 * (t + 1)], idn_sb[:]
                    )
                for i in range(4):
                    t = 4 * tq + i
                    nc.vector.tensor_copy(out=vp_sb[:, t, 0:64], in_=vt_ps[:, i, :])

            o_ps = psO.tile([H + 1, TQ], fp32, tag="o")

            def emit_p(p):
                """One local k-tile: S^T matmul, exp (+mask), AV accumulate."""
                a0 = 128 * (p % 8)
                pieces = [(a0, 512), (512, 1024)] if a0 < 512 else [(a0, 1024)]
                for (lo, hi) in pieces:
                    w = hi - lo
                    s_ps = psS.tile([128, 512], fp32, tag="s")
                    nc.tensor.matmul(
                        s_ps[:, 0:w],
                        kT_sb[:, 128 * p:128 * (p + 1)],
                        qT_sb[:, lo:hi],
                        start=True, stop=True,
                    )
                    pt = ptp.tile([128, 512], bf16, tag="pt")
                    if lo == a0 and p >= 8:
                        # boundary block: valid only for r==1 (gate bias)
                        nc.scalar.activation(
                            pt[:, 0:128], s_ps[:, 0:128], Exp,
                            bias=gate_sb[:, 0:1], scale=SCALE,
                        )
                        if w > 128:
                            nc.scalar.activation(
                                pt[:, 128:w], s_ps[:, 128:w], Exp, scale=SCALE
                            )
                    else:
                        nc.scalar.activation(pt[:, 0:w], s_ps[:, 0:w], Exp, scale=SCALE)
                        if lo == a0 and p < 8:
                            # diagonal block: constant triangular mask
                            nc.vector.tensor_mul(pt[:, 0:128], pt[:, 0:128], tri_sb[:])
                    nc.tensor.matmul(
                        o_ps[:, lo:hi],
                        vp_sb[:, p, 0:65],
                        pt[:, 0:w],
                        start=(p == 0), stop=(p == NKT - 1),
                        skip_group_check=True,
                    )

            # own half: project, transpose v, attend (runs while the other
            # half of x^T is still in flight on the gpsimd queue)
            emit_quarter(0)
            emit_vtrans(0)
            emit_quarter(1)
            emit_vtrans(1)
            for p in range(8):
                emit_p(p)

            # other half
            emit_quarter(2)
            emit_vtrans(2)
            emit_quarter(3)
            emit_vtrans(3)
            for p in range(8, 12):
                emit_p(p)
            # cols 0:512 fully accumulated after p=11 -> drain early
            nc.vector.tensor_copy(out=o_sb[:, 0:512], in_=o_ps[:, 0:512])
            nc.sync.dma_start(out=out_d[:, 0:512], in_=o_sb[:, 0:512])
            for p in range(12, NKT):
                emit_p(p)
            nc.vector.tensor_copy(out=o_sb[:, 512:1024], in_=o_ps[:, 512:1024])
            nc.sync.dma_start(out=out_d[:, 512:1024], in_=o_sb[:, 512:1024])

    nc.finalize()
    return nc


def _get_program():
    if "nc" not in _prog_cache:
        _prog_cache["nc"] = _build_program()
    return _prog_cache["nc"]


def make_in_maps(x, Wq, Wk, Wv):
    bf16 = ml_dtypes.bfloat16
    wqk = np.concatenate([Wq, Wk], axis=1).astype(bf16)  # [C, 128]
    wv = np.ascontiguousarray(Wv.astype(bf16))
    tri = np.triu(np.ones((128, 128), np.float32)).astype(bf16)  # tri[k,q]=1 iff q>=k
    idn = np.eye(64, dtype=np.float32).astype(bf16)
    in_maps = []
    for core in range(8):
        b, r = core // 2, core % 2
        own = [2 * m + r for m in range(8)]
        other = [2 * m + 1 - r for m in range(8)]
        idx = np.concatenate([np.arange(g * 128, (g + 1) * 128) for g in own + other])
        xt = np.asarray(x[b]).T[:, idx]
        gate = np.full((128, 1), 0.0 if r == 1 else -60.0, np.float32)
        in_maps.append({
            "xt": np.ascontiguousarray(xt).astype(bf16),
            "wqk": wqk,
            "wv": wv,
            "gate": gate,
            "tri": tri,
            "idn": idn,
        })
    return in_maps


def postprocess(results):
    out = np.empty((B, T, H), np.float32)
    for core in range(8):
        b, r = core // 2, core % 2
        oT = results[core]["outT"]  # [65, 1024]
        vals = (oT[:H] / oT[H:H + 1]).T.reshape(8, 128, H)
        ob = out[b].reshape(16, 128, H)
        for m in range(8):
            ob[2 * m + r] = vals[m]
    return out


def kernel(x, mask, Wq, Wk, Wv, _trace=False, _tracedir=None):
    from concourse import bass_utils

    nc = _get_program()
    in_maps = make_in_maps(np.asarray(x, np.float32), np.asarray(Wq, np.float32),
                           np.asarray(Wk, np.float32), np.asarray(Wv, np.float32))
    res = bass_utils.run_bass_kernel_spmd(
        nc, in_maps, core_ids=list(range(8)),
        trace=_trace, tmpdir=_tracedir,
    )
    out = postprocess(res.results)
    if _trace:
        return out, res
    return out


# revision 4
# speedup vs baseline: 1.2380x; 1.0122x over previous
"""Trainium2 Bass kernel for single-head causal attention.

Problem: x[B=4,T=2048,C=1024] -> q,k,v = x@Wq/Wk/Wv [T,64] -> causal softmax(q k^T/sqrt(C)) @ v.

Sharding: 8 cores = 4 batches x 2 interleaved query-tile sets. Core r of a
batch owns global 128-row q-tiles {2m+r : m=0..7} -- interleaving balances
the causal triangle exactly (68 vs 68 blocks) instead of 36 vs 100 for
contiguous halves.

SPMD-uniform trick: each core's x^T copy is column-permuted so its OWN
q-tiles come first (local tiles 0-7), the peer's after (local 8-15). The
causal block structure is then core-independent:
  - local k-tile p<8  (own tiles, global 2p+r): q-window m >= p; the m==p
    block is the diagonal -> multiply by a constant 128x128 triangle.
  - local k-tile p>=8 (peer tiles, global 2(p-8)+1-r): q-window m >= p-8;
    the boundary block m==p-8 is valid only for r==1, gated by a per-core
    exp bias (0 keeps, -60 zeroes). All other blocks are fully valid.
Softmax normalization is fused into the AV matmul by appending a ones
column to V (output row 64 = sum of exp); division happens host-side.

Perf structure:
  - S^T = k_tile-stationary x q^T-moving in bf16 (4x faster than fp32 on
    the PE); P^T feeds AV with V-natural stationary (+ones row for l).
  - V transposed on the PE (identity transpose), not DMA-xbar.
  - x^T shipped half-major/chunk-major so each DMA moves 4 contraction
    chunks with 8KB-per-partition descriptors; constants packed into one
    contiguous [128, 1728] block. Own half lands first, split across the
    sync and scalar HWDGE queues.
  - Projection runs quarter-PAIRS interleaved per chunk so it tracks DMA
    arrival; own-half attention (S/exp/AV) runs before the other half's
    projection, which keeps the ACT engine (exp is co-critical) busy from
    the earliest possible moment.
"""

import numpy as np
import ml_dtypes

B, T, C, H = 4, 2048, 1024, 64
TQ = 1024          # queries per core
NT = 2048          # kv length per core
NCH = C // 128     # 8 contraction chunks
NKT = NT // 128    # 16 local k-tiles
SCALE = 1.0 / 32.0  # 1/sqrt(C)
VSTRIDE = 80       # bf16 cols per v' tile slot (64 v + 1 ones + pad)
CW = 1728          # packed constant cols: 1024 wqk + 512 wv + 128 tri + 64 idn

_prog_cache = {}


def _build_program():
    import concourse.mybir as mybir
    from concourse import bacc
    from concourse.tile import TileContext

    fp32 = mybir.dt.float32
    bf16 = mybir.dt.bfloat16
    Exp = mybir.ActivationFunctionType.Exp

    nc = bacc.Bacc("TRN2", target_bir_lowering=False, debug=False)

    xt_d = nc.dram_tensor("xt", [128, 2, NCH, TQ], bf16, kind="ExternalInput")
    cst_d = nc.dram_tensor("cst", [128, CW], bf16, kind="ExternalInput")
    gate_d = nc.dram_tensor("gate", [128, 1], fp32, kind="ExternalInput")
    out_d = nc.dram_tensor("outT", [H + 1, TQ], fp32, kind="ExternalOutput")

    with TileContext(nc) as tc:
        with (
            tc.tile_pool(name="xtp", bufs=1) as xt_pool,
            tc.tile_pool(name="cstp", bufs=1) as cstp,
            tc.tile_pool(name="prj", bufs=1) as prj,
            tc.tile_pool(name="ptp", bufs=4) as ptp,
            tc.tile_pool(name="psA", bufs=2, space="PSUM") as psA,
            tc.tile_pool(name="psB", bufs=2, space="PSUM") as psB,
            tc.tile_pool(name="psS", bufs=2, space="PSUM") as psS,
            tc.tile_pool(name="psO", bufs=1, space="PSUM") as psO,
        ):
            # packed constants (single contiguous DMA, 3.4KB descriptors)
            cst_sb = cstp.tile([128, CW], bf16, tag="cst")
            nc.sync.dma_start(out=cst_sb[:], in_=cst_d[:])
            gate_sb = cstp.tile([128, 1], fp32, tag="gate")
            nc.scalar.dma_start(out=gate_sb[:], in_=gate_d[:])

            def wqk(c):
                return cst_sb[:, c * 128:(c + 1) * 128]

            def wv(c):
                return cst_sb[:, 1024 + c * 64:1024 + (c + 1) * 64]

            tri_sb = cst_sb[:, 1536:1664]
            idn_sb = cst_sb[0:64, 1664:1728]

            # x^T, half-major then chunk-major: [128, half, chunk, time]
            # 2 quad-chunk DMAs per half per queue -> 8KB descriptors
            xt_sb = prj.tile([128, 2, NCH, TQ], bf16, tag="xt")
            nc.sync.dma_start(out=xt_sb[:, 0, 0:4, :], in_=xt_d[:, 0, 0:4, :])
            nc.scalar.dma_start(out=xt_sb[:, 0, 4:8, :], in_=xt_d[:, 0, 4:8, :])
            nc.sync.dma_start(out=xt_sb[:, 1, 0:4, :], in_=xt_d[:, 1, 0:4, :])
            nc.scalar.dma_start(out=xt_sb[:, 1, 4:8, :], in_=xt_d[:, 1, 4:8, :])

            # persistent projection outputs (all bf16)
            qT_sb = prj.tile([64, TQ], bf16, tag="qT")
            kT_sb = prj.tile([64, NT], bf16, tag="kT")
            vT_sb = prj.tile([64, NT], bf16, tag="vT")
            vp_sb = prj.tile([128, NKT, VSTRIDE], bf16, tag="vp")
            o_sb = prj.tile([H + 1, TQ], fp32, tag="osb")
            warm_sb = prj.tile([128, 64], bf16, tag="warm")

            # ones column (col 64 of each VSTRIDE block) for the l-row trick
            nc.vector.memset(vp_sb[:, :, 64:65], 1.0)

            # PE p-state warmup during the initial x^T DMA (result copied
            # out so the verifier sees a reader)
            warm_ps = psS.tile([128, 512], fp32, tag="s")
            for _ in range(6):
                nc.tensor.matmul(
                    warm_ps[:], wqk(0), cst_sb[:, 0:512],
                    start=True, stop=True,
                )
            nc.vector.tensor_copy(out=warm_sb[:, 0:64], in_=warm_ps[:, 0:64])
            # warm the Exp activation table early (1.3us load off the
            # critical path)
            nc.scalar.activation(warm_sb[0:128, 0:1], warm_ps[:, 0:1], Exp, scale=SCALE)

            def emit_half_proj(h):
                """Project half h: q^T/k^T (packed) and v^T for local tiles
                8h..8h+7, quarter-pair interleaved per contraction chunk so
                compute tracks DMA arrival."""
                qk_ps = [psA.tile([128, 512], fp32, tag="qk", name=f"qk{h}{i}") for i in range(2)]
                v_ps = [psB.tile([64, 512], fp32, tag="pv", name=f"pv{h}{i}") for i in range(2)]
                for c in range(NCH):
                    mv = xt_sb[:, h, c, :]
                    for i in range(2):
                        nc.tensor.matmul(
                            qk_ps[i][:], wqk(c), mv[:, 512 * i:512 * (i + 1)],
                            start=(c == 0), stop=(c == NCH - 1),
                        )
                    for i in range(2):
                        nc.tensor.matmul(
                            v_ps[i][:], wv(c), mv[:, 512 * i:512 * (i + 1)],
                            start=(c == 0), stop=(c == NCH - 1),
                        )
                for i in range(2):
                    sl = slice(1024 * h + 512 * i, 1024 * h + 512 * (i + 1))
                    if h == 0:
                        nc.vector.tensor_copy(out=qT_sb[:, 512 * i:512 * (i + 1)], in_=qk_ps[i][0:64, :])
                    nc.vector.tensor_copy(out=kT_sb[:, sl], in_=qk_ps[i][64:128, :])
                    nc.vector.tensor_copy(out=vT_sb[:, sl], in_=v_ps[i][:])

            def emit_vtrans(tq):
                """PE-transpose v^T tiles 4*tq..4*tq+3 into vp (v-natural)."""
                vt_ps = psS.tile([128, 8, 64], bf16, tag="s", name=f"vt{tq}")
                for i in range(4):
                    t = 4 * tq + i
                    nc.tensor.transpose(
                        vt_ps[:, i, :], vT_sb[:, 128 * t:128 * (t + 1)], idn_sb
                    )
                for i in range(4):
                    t = 4 * tq + i
                    nc.vector.tensor_copy(out=vp_sb[:, t, 0:64], in_=vt_ps[:, i, :])

            o_ps = psO.tile([H + 1, TQ], fp32, tag="o")

            def emit_p(p):
                """One local k-tile: S^T matmul, exp (+mask), AV accumulate."""
                a0 = 128 * (p % 8)
                pieces = [(a0, 512), (512, 1024)] if a0 < 512 else [(a0, 1024)]
                for (lo, hi) in pieces:
                    w = hi - lo
                    s_ps = psS.tile([128, 512], fp32, tag="s", name=f"s{p}_{lo}")
                    nc.tensor.matmul(
                        s_ps[:, 0:w],
                        kT_sb[:, 128 * p:128 * (p + 1)],
                        qT_sb[:, lo:hi],
                        start=True, stop=True,
                    )
                    pt = ptp.tile([128, 512], bf16, tag="pt", name=f"pt{p}_{lo}")
                    if lo == a0 and p >= 8:
                        # boundary block: valid only for r==1 (gate bias)
                        nc.scalar.activation(
                            pt[:, 0:128], s_ps[:, 0:128], Exp,
                            bias=gate_sb[:, 0:1], scale=SCALE,
                        )
                        if w > 128:
                            nc.scalar.activation(
                                pt[:, 128:w], s_ps[:, 128:w], Exp, scale=SCALE
                            )
                    else:
                        nc.scalar.activation(pt[:, 0:w], s_ps[:, 0:w], Exp, scale=SCALE)
                        if lo == a0 and p < 8:
                            # diagonal block: constant triangular mask
                            nc.vector.tensor_mul(pt[:, 0:128], pt[:, 0:128], tri_sb)
                    nc.tensor.matmul(
                        o_ps[:, lo:hi],
                        vp_sb[:, p, 0:65],
                        pt[:, 0:w],
                        start=(p == 0), stop=(p == NKT - 1),
                        skip_group_check=True,
                    )

            # own half: project, transpose v, attend. The other half of x^T
            # is still in flight during attention; its projection runs after
            # own-half attention is emitted so the PE never blocks on it.
            emit_half_proj(0)
            emit_vtrans(0)
            emit_vtrans(1)
            for p in range(8):
                emit_p(p)

            # other half
            emit_half_proj(1)
            emit_vtrans(2)
            emit_vtrans(3)
            for p in range(8, 12):
                emit_p(p)
            # cols 0:512 fully accumulated after p=11 -> drain early
            nc.vector.tensor_copy(out=o_sb[:, 0:512], in_=o_ps[:, 0:512])
            nc.sync.dma_start(out=out_d[:, 0:512], in_=o_sb[:, 0:512])
            for p in range(12, NKT):
                emit_p(p)
            nc.vector.tensor_copy(out=o_sb[:, 512:1024], in_=o_ps[:, 512:1024])
            nc.sync.dma_start(out=out_d[:, 512:1024], in_=o_sb[:, 512:1024])

    nc.finalize()
    return nc


def _get_program():
    if "nc" not in _prog_cache:
        _prog_cache["nc"] = _build_program()
    return _prog_cache["nc"]


def make_in_maps(x, Wq, Wk, Wv):
    bf16 = ml_dtypes.bfloat16
    wqk = np.concatenate([Wq, Wk], axis=1)          # [C, 128]
    wqk_p = wqk.reshape(8, 128, 128).transpose(1, 0, 2).reshape(128, 1024)
    wv_p = np.asarray(Wv).reshape(8, 128, 64).transpose(1, 0, 2).reshape(128, 512)
    tri = np.triu(np.ones((128, 128), np.float32))  # tri[k,q]=1 iff q>=k
    idn = np.zeros((128, 64), np.float32)
    idn[:64] = np.eye(64, dtype=np.float32)
    cst = np.ascontiguousarray(
        np.concatenate([wqk_p, wv_p, tri, idn], axis=1)
    ).astype(bf16)
    in_maps = []
    for core in range(8):
        b, r = core // 2, core % 2
        own = [2 * m + r for m in range(8)]
        other = [2 * m + 1 - r for m in range(8)]
        idx = np.concatenate([np.arange(g * 128, (g + 1) * 128) for g in own + other])
        xp = np.asarray(x[b]).T[:, idx]             # [C, 2048] permuted
        xt = xp.reshape(8, 128, 2, 1024).transpose(1, 2, 0, 3)  # [128,2,8,1024]
        gate = np.full((128, 1), 0.0 if r == 1 else -60.0, np.float32)
        in_maps.append({
            "xt": np.ascontiguousarray(xt).astype(bf16),
            "cst": cst,
            "gate": gate,
        })
    return in_maps


def postprocess(results):
    out = np.empty((B, T, H), np.float32)
    for core in range(8):
        b, r = core // 2, core % 2
        oT = results[core]["outT"]  # [65, 1024]
        vals = (oT[:H] / oT[H:H + 1]).T.reshape(8, 128, H)
        ob = out[b].reshape(16, 128, H)
        for m in range(8):
            ob[2 * m + r] = vals[m]
    return out


def kernel(x, mask, Wq, Wk, Wv, _trace=False, _tracedir=None):
    from concourse import bass_utils

    nc = _get_program()
    in_maps = make_in_maps(np.asarray(x, np.float32), np.asarray(Wq, np.float32),
                           np.asarray(Wk, np.float32), np.asarray(Wv, np.float32))
    res = bass_utils.run_bass_kernel_spmd(
        nc, in_maps, core_ids=list(range(8)),
        trace=_trace, tmpdir=_tracedir,
    )
    out = postprocess(res.results)
    if _trace:
        return out, res
    return out


# revision 6
# speedup vs baseline: 1.2932x; 1.0445x over previous
"""Trainium2 Bass kernel for single-head causal attention.

Problem: x[B=4,T=2048,C=1024] -> q,k,v = x@Wq/Wk/Wv [T,64] -> causal softmax(q k^T/sqrt(C)) @ v.

Sharding: 8 cores = 4 batches x 2 interleaved query-tile sets. Core r of a
batch owns global 128-row q-tiles {2m+r : m=0..7} -- interleaving balances
the causal triangle exactly (68 vs 68 blocks) instead of 36 vs 100 for
contiguous halves.

SPMD-uniform trick: each core's x^T copy is column-permuted so its OWN
q-tiles come first (local tiles 0-7), the peer's after (local 8-15). The
causal block structure is then core-independent:
  - local k-tile p<8  (own tiles, global 2p+r): q-window m >= p; the m==p
    block is the diagonal -> multiply by a constant 128x128 triangle.
  - local k-tile p>=8 (peer tiles, global 2(p-8)+1-r): q-window m >= p-8;
    the boundary block m==p-8 is valid only for r==1 -> multiply by a
    per-core all-ones/all-zeros mask. All other blocks are fully valid.
Softmax normalization is fused into the AV matmul by appending a ones
column to V (output row 64 = sum of exp); division happens host-side.

Perf structure:
  - x^T and W in bf16 (fp8 was tried and fails the accuracy gate: V-path
    quantization noise lands ~1:1 in the output).
  - S^T = k_tile-stationary x q^T-moving in bf16; P^T feeds AV with
    V-natural stationary (+ones row for the softmax denominator).
  - V transposed on the PE (identity transpose), not DMA-xbar.
  - Boundary-block masks (triangle / gate) are tensor_muls on the
    otherwise-idle GPSIMD engine; exp is one ACT instruction per piece.
  - DMA: two HWDGE queues (sync+scalar), own half first, 8KB descriptors;
    weights packed into one contiguous block per phase.
"""

import numpy as np
import ml_dtypes

B, T, C, H = 4, 2048, 1024, 64
TQ = 1024          # queries per core
NT = 2048          # kv length per core
NCH = C // 128     # 8 contraction chunks
NKT = NT // 128    # 16 local k-tiles
SCALE = 1.0 / 32.0  # 1/sqrt(C)
VSTRIDE = 80       # bf16 cols per v' tile slot (64 v + 1 ones + pad)
CWA = 1536         # packed fp8 weight cols: 1024 wqk + 512 wv
CWB = 320          # packed bf16 const cols: 128 tri + 64 idn + 128 gmask

_prog_cache = {}


def _build_program():
    import concourse.mybir as mybir
    from concourse import bacc
    from concourse.tile import TileContext

    fp32 = mybir.dt.float32
    bf16 = mybir.dt.bfloat16
    Exp = mybir.ActivationFunctionType.Exp

    nc = bacc.Bacc("TRN2", target_bir_lowering=False, debug=False)

    xt_d = nc.dram_tensor("xt", [128, 2, NCH, TQ], bf16, kind="ExternalInput")
    cstA_d = nc.dram_tensor("cstA", [128, CWA], bf16, kind="ExternalInput")
    cstB_d = nc.dram_tensor("cstB", [128, CWB], bf16, kind="ExternalInput")
    out_d = nc.dram_tensor("outT", [H + 1, TQ], fp32, kind="ExternalOutput")

    with TileContext(nc) as tc:
        with (
            tc.tile_pool(name="cstp", bufs=1) as cstp,
            tc.tile_pool(name="prj", bufs=1) as prj,
            tc.tile_pool(name="ptp", bufs=4) as ptp,
            tc.tile_pool(name="psA", bufs=2, space="PSUM") as psA,
            tc.tile_pool(name="psB", bufs=2, space="PSUM") as psB,
            tc.tile_pool(name="psS", bufs=2, space="PSUM") as psS,
            tc.tile_pool(name="psO", bufs=1, space="PSUM") as psO,
        ):
            # DMA plan: sync queue gets x^T quads 0-3 (own first), scalar
            # queue gets the weights then x^T quads 4-7.
            xt_sb = prj.tile([128, 2, NCH, TQ], bf16, tag="xt")
            cstA_sb = cstp.tile([128, CWA], bf16, tag="cstA")
            cstB_sb = cstp.tile([128, CWB], bf16, tag="cstB")
            nc.sync.dma_start(out=xt_sb[:, 0, 0:4, :], in_=xt_d[:, 0, 0:4, :])
            nc.scalar.dma_start(out=cstA_sb[:], in_=cstA_d[:])
            nc.scalar.dma_start(out=xt_sb[:, 0, 4:8, :], in_=xt_d[:, 0, 4:8, :])
            nc.sync.dma_start(out=xt_sb[:, 1, 0:4, :], in_=xt_d[:, 1, 0:4, :])
            nc.scalar.dma_start(out=cstB_sb[:], in_=cstB_d[:])
            nc.scalar.dma_start(out=xt_sb[:, 1, 4:8, :], in_=xt_d[:, 1, 4:8, :])

            def wqk(c):
                return cstA_sb[:, c * 128:(c + 1) * 128]

            def wv(c):
                return cstA_sb[:, 1024 + c * 64:1024 + (c + 1) * 64]

            tri_sb = cstB_sb[:, 0:128]
            idn_sb = cstB_sb[0:64, 128:192]
            gm_sb = cstB_sb[:, 192:320]

            # persistent projection outputs
            qT_sb = prj.tile([64, TQ], bf16, tag="qT")
            kT_sb = prj.tile([64, NT], bf16, tag="kT")
            vT_sb = prj.tile([64, NT], bf16, tag="vT")
            vp_sb = prj.tile([128, NKT, VSTRIDE], bf16, tag="vp")
            o_sb = prj.tile([H + 1, TQ], fp32, tag="osb")
            warm_sb = prj.tile([128, 512], bf16, tag="warm")

            # ones column (col 64 of each VSTRIDE block) for the l-row trick
            nc.vector.memset(vp_sb[:, :, 64:65], 1.0)
            nc.vector.memset(warm_sb[:, 0:128], 0.0)

            # PE p-state warmup on a local dummy tile -- starts as soon as
            # the memset lands, no DMA dependency. Result copied out so the
            # verifier sees a reader.
            warm_ps = psS.tile([128, 512], fp32, tag="s")
            for _ in range(6):
                nc.tensor.matmul(
                    warm_ps[:], warm_sb[:, 0:128], warm_sb[:, 0:512],
                    start=True, stop=True,
                )
            nc.vector.tensor_copy(out=warm_sb[:, 0:64], in_=warm_ps[:, 0:64])
            # warm the Exp activation table early (1.3us load off the
            # critical path)
            nc.scalar.activation(warm_sb[0:128, 0:1], warm_ps[:, 0:1], Exp, scale=SCALE)

            def emit_half_proj(h):
                """Project half h: q^T/k^T (packed) and v^T for local tiles
                8h..8h+7, quarter-pair interleaved per contraction chunk so
                compute tracks DMA arrival."""
                qk_ps = [psA.tile([128, 512], fp32, tag="qk", name=f"qk{h}{i}") for i in range(2)]
                v_ps = [psB.tile([64, 512], fp32, tag="pv", name=f"pv{h}{i}") for i in range(2)]
                for c in range(NCH):
                    mv = xt_sb[:, h, c, :]
                    for i in range(2):
                        nc.tensor.matmul(
                            qk_ps[i][:], wqk(c), mv[:, 512 * i:512 * (i + 1)],
                            start=(c == 0), stop=(c == NCH - 1),
                        )
                    for i in range(2):
                        nc.tensor.matmul(
                            v_ps[i][:], wv(c), mv[:, 512 * i:512 * (i + 1)],
                            start=(c == 0), stop=(c == NCH - 1),
                        )
                for i in range(2):
                    sl = slice(1024 * h + 512 * i, 1024 * h + 512 * (i + 1))
                    if h == 0:
                        nc.vector.tensor_copy(out=qT_sb[:, 512 * i:512 * (i + 1)], in_=qk_ps[i][0:64, :])
                    nc.vector.tensor_copy(out=kT_sb[:, sl], in_=qk_ps[i][64:128, :])
                    nc.vector.tensor_copy(out=vT_sb[:, sl], in_=v_ps[i][:])

            def emit_vtrans(tq):
                """PE-transpose v^T tiles 4*tq..4*tq+3 into vp (v-natural)."""
                vt_ps = psS.tile([128, 8, 64], bf16, tag="s", name=f"vt{tq}")
                for i in range(4):
                    t = 4 * tq + i
                    nc.tensor.transpose(
                        vt_ps[:, i, :], vT_sb[:, 128 * t:128 * (t + 1)], idn_sb
                    )
                for i in range(4):
                    t = 4 * tq + i
                    nc.vector.tensor_copy(out=vp_sb[:, t, 0:64], in_=vt_ps[:, i, :])

            o_ps = psO.tile([H + 1, TQ], fp32, tag="o")

            def emit_p(p):
                """One local k-tile: S^T matmul, exp, boundary mask (GPSIMD),
                AV accumulate."""
                a0 = 128 * (p % 8)
                pieces = [(a0, 512), (512, 1024)] if a0 < 512 else [(a0, 1024)]
                mask = tri_sb if p < 8 else gm_sb
                for (lo, hi) in pieces:
                    w = hi - lo
                    s_ps = psS.tile([128, 512], fp32, tag="s", name=f"s{p}_{lo}")
                    nc.tensor.matmul(
                        s_ps[:, 0:w],
                        kT_sb[:, 128 * p:128 * (p + 1)],
                        qT_sb[:, lo:hi],
                        start=True, stop=True,
                    )
                    pt = ptp.tile([128, 512], bf16, tag="pt", name=f"pt{p}_{lo}")
                    nc.scalar.activation(pt[:, 0:w], s_ps[:, 0:w], Exp, scale=SCALE)
                    if lo == a0:
                        nc.gpsimd.tensor_mul(pt[:, 0:128], pt[:, 0:128], mask)
                    nc.tensor.matmul(
                        o_ps[:, lo:hi],
                        vp_sb[:, p, 0:65],
                        pt[:, 0:w],
                        start=(p == 0), stop=(p == NKT - 1),
                        skip_group_check=True,
                    )

            # own half: project, transpose v, attend. The other half of x^T
            # is still in flight during own attention.
            emit_half_proj(0)
            emit_vtrans(0)
            emit_vtrans(1)
            for p in range(8):
                emit_p(p)

            # other half
            emit_half_proj(1)
            emit_vtrans(2)
            emit_vtrans(3)
            for p in range(8, 12):
                emit_p(p)
            # cols 0:512 fully accumulated after p=11 -> drain early
            nc.vector.tensor_copy(out=o_sb[:, 0:512], in_=o_ps[:, 0:512])
            nc.sync.dma_start(out=out_d[:, 0:512], in_=o_sb[:, 0:512])
            for p in range(12, NKT):
                emit_p(p)
            nc.vector.tensor_copy(out=o_sb[:, 512:1024], in_=o_ps[:, 512:1024])
            nc.sync.dma_start(out=out_d[:, 512:1024], in_=o_sb[:, 512:1024])

    nc.finalize()
    return nc


def _get_program():
    if "nc" not in _prog_cache:
        _prog_cache["nc"] = _build_program()
    return _prog_cache["nc"]


def make_in_maps(x, Wq, Wk, Wv):
    bf16 = ml_dtypes.bfloat16
    wqk = np.concatenate([Wq, Wk], axis=1)          # [C, 128]
    wqk_p = wqk.reshape(8, 128, 128).transpose(1, 0, 2).reshape(128, 1024)
    wv_p = np.asarray(Wv).reshape(8, 128, 64).transpose(1, 0, 2).reshape(128, 512)
    cstA = np.ascontiguousarray(np.concatenate([wqk_p, wv_p], axis=1)).astype(bf16)
    tri = np.triu(np.ones((128, 128), np.float32))  # tri[k,q]=1 iff q>=k
    idn = np.zeros((128, 64), np.float32)
    idn[:64] = np.eye(64, dtype=np.float32)
    in_maps = []
    for core in range(8):
        b, r = core // 2, core % 2
        own = [2 * m + r for m in range(8)]
        other = [2 * m + 1 - r for m in range(8)]
        idx = np.concatenate([np.arange(g * 128, (g + 1) * 128) for g in own + other])
        xp = np.asarray(x[b]).T[:, idx]             # [C, 2048] permuted
        xt = xp.reshape(8, 128, 2, 1024).transpose(1, 2, 0, 3)  # [128,2,8,1024]
        gm = np.full((128, 128), 1.0 if r == 1 else 0.0, np.float32)
        cstB = np.ascontiguousarray(
            np.concatenate([tri, idn, gm], axis=1)
        ).astype(bf16)
        in_maps.append({
            "xt": np.ascontiguousarray(xt).astype(bf16),
            "cstA": cstA,
            "cstB": cstB,
        })
    return in_maps


def postprocess(results):
    out = np.empty((B, T, H), np.float32)
    for core in range(8):
        b, r = core // 2, core % 2
        oT = results[core]["outT"]  # [65, 1024]
        vals = (oT[:H] / oT[H:H + 1]).T.reshape(8, 128, H)
        ob = out[b].reshape(16, 128, H)
        for m in range(8):
            ob[2 * m + r] = vals[m]
    return out


def kernel(x, mask, Wq, Wk, Wv, _trace=False, _tracedir=None):
    from concourse import bass_utils

    nc = _get_program()
    in_maps = make_in_maps(np.asarray(x, np.float32), np.asarray(Wq, np.float32),
                           np.asarray(Wk, np.float32), np.asarray(Wv, np.float32))
    res = bass_utils.run_bass_kernel_spmd(
        nc, in_maps, core_ids=list(range(8)),
        trace=_trace, tmpdir=_tracedir,
    )
    out = postprocess(res.results)
    if _trace:
        return out, res
    return out


# revision 8
# speedup vs baseline: 1.3768x; 1.0647x over previous
"""Trainium2 Bass kernel for single-head causal attention.

Problem: x[B=4,T=2048,C=1024] -> q,k,v = x@Wq/Wk/Wv [T,64] -> causal softmax(q k^T/sqrt(C)) @ v.

Sharding: 8 cores = 4 batches x 2 interleaved query-tile sets. Core r of a
batch owns global 128-row q-tiles {2m+r : m=0..7} -- interleaving balances
the causal triangle exactly (68 vs 68 blocks) instead of 36 vs 100 for
contiguous halves.

SPMD-uniform trick: each core's x^T copy is column-permuted so its OWN
q-tiles come first (local tiles 0-7), the peer's after (local 8-15). The
causal block structure is then core-independent:
  - local k-tile p<8  (own tiles, global 2p+r): q-window m >= p; the m==p
    block is the diagonal -> multiply by a constant 128x128 triangle.
  - local k-tile p>=8 (peer tiles, global 2(p-8)+1-r): q-window m >= p-8;
    the boundary block m==p-8 is valid only for r==1 -> multiply by a
    per-core all-ones/all-zeros mask. All other blocks are fully valid.
Softmax normalization is fused into the AV matmul by appending a ones
column to V (output row 64 = sum of exp); division happens host-side.

Perf structure:
  - x^T and W in bf16 (fp8 was tried and fails the accuracy gate: V-path
    quantization noise lands ~1:1 in the output).
  - S^T = k_tile-stationary x q^T-moving in bf16; P^T feeds AV with
    V-natural stationary (+ones row for the softmax denominator).
  - V transposed on the PE (identity transpose), not DMA-xbar.
  - Boundary-block masks (triangle / gate) are tensor_muls on the
    otherwise-idle GPSIMD engine; exp is one ACT instruction per piece.
  - DMA: two HWDGE queues (sync+scalar), own half first, 8KB descriptors;
    weights packed into one contiguous block per phase.
"""

import numpy as np
import ml_dtypes

B, T, C, H = 4, 2048, 1024, 64
TQ = 1024          # queries per core
NT = 2048          # kv length per core
NCH = C // 128     # 8 contraction chunks
NKT = NT // 128    # 16 local k-tiles
SCALE = 1.0 / 32.0  # 1/sqrt(C)
VSTRIDE = 80       # bf16 cols per v' tile slot (64 v + 1 ones + pad)
CWA = 1536         # packed fp8 weight cols: 1024 wqk + 512 wv
CWB = 320          # packed bf16 const cols: 128 tri + 64 idn + 128 gmask

_prog_cache = {}


def _build_program():
    import concourse.mybir as mybir
    from concourse import bacc
    from concourse.tile import TileContext

    fp32 = mybir.dt.float32
    bf16 = mybir.dt.bfloat16
    Exp = mybir.ActivationFunctionType.Exp

    nc = bacc.Bacc("TRN2", target_bir_lowering=False, debug=False)

    xt_d = nc.dram_tensor("xt", [128, 2, NCH, TQ], bf16, kind="ExternalInput")
    cstA_d = nc.dram_tensor("cstA", [128, CWA], bf16, kind="ExternalInput")
    cstB_d = nc.dram_tensor("cstB", [128, CWB], bf16, kind="ExternalInput")
    out_d = nc.dram_tensor("outT", [H + 1, TQ], fp32, kind="ExternalOutput")

    with TileContext(nc) as tc:
        with (
            tc.tile_pool(name="cstp", bufs=1) as cstp,
            tc.tile_pool(name="prj", bufs=1) as prj,
            tc.tile_pool(name="ptp", bufs=4) as ptp,
            tc.tile_pool(name="psX", bufs=4, space="PSUM") as psX,
            tc.tile_pool(name="psB", bufs=2, space="PSUM") as psB,
            tc.tile_pool(name="psO", bufs=1, space="PSUM") as psO,
        ):
            # DMA plan: sync queue gets x^T quads 0-3 (own first), scalar
            # queue gets the weights then x^T quads 4-7.
            xt_sb = prj.tile([128, 2, NCH, TQ], bf16, tag="xt")
            cstA_sb = cstp.tile([128, CWA], bf16, tag="cstA")
            cstB_sb = cstp.tile([128, CWB], bf16, tag="cstB")
            nc.sync.dma_start(out=xt_sb[:, 0, 0:4, :], in_=xt_d[:, 0, 0:4, :])
            nc.scalar.dma_start(out=cstA_sb[:], in_=cstA_d[:])
            nc.scalar.dma_start(out=xt_sb[:, 0, 4:8, :], in_=xt_d[:, 0, 4:8, :])
            nc.sync.dma_start(out=xt_sb[:, 1, 0:4, :], in_=xt_d[:, 1, 0:4, :])
            nc.scalar.dma_start(out=cstB_sb[:], in_=cstB_d[:])
            nc.scalar.dma_start(out=xt_sb[:, 1, 4:8, :], in_=xt_d[:, 1, 4:8, :])

            def wqk(c):
                return cstA_sb[:, c * 128:(c + 1) * 128]

            def wv(c):
                return cstA_sb[:, 1024 + c * 64:1024 + (c + 1) * 64]

            tri_sb = cstB_sb[:, 0:128]
            idn_sb = cstB_sb[0:64, 128:192]
            gm_sb = cstB_sb[:, 192:320]

            # persistent projection outputs
            qT_sb = prj.tile([64, TQ], bf16, tag="qT")
            kT_sb = prj.tile([64, NT], bf16, tag="kT")
            vT_sb = prj.tile([64, NT], bf16, tag="vT")
            vp_sb = prj.tile([128, NKT, VSTRIDE], bf16, tag="vp")
            o_sb = prj.tile([H + 1, TQ], fp32, tag="osb")
            warm_sb = prj.tile([128, 512], bf16, tag="warm")

            # ones column (col 64 of each VSTRIDE block) for the l-row trick
            nc.vector.memset(vp_sb[:, :, 64:65], 1.0)
            nc.vector.memset(warm_sb[:, 0:128], 0.0)

            # PE p-state warmup on a local dummy tile -- starts as soon as
            # the memset lands, no DMA dependency. Result copied out so the
            # verifier sees a reader.
            warm_ps = psX.tile([128, 512], fp32, tag="x")
            for _ in range(8):
                nc.tensor.matmul(
                    warm_ps[:], warm_sb[:, 0:128], warm_sb[:, 0:512],
                    start=True, stop=True,
                )
            nc.vector.tensor_copy(out=warm_sb[:, 0:64], in_=warm_ps[:, 0:64])
            # warm the Exp activation table early (1.3us load off the
            # critical path)
            nc.scalar.activation(warm_sb[0:128, 0:1], warm_ps[:, 0:1], Exp, scale=SCALE)

            def emit_half_proj(h):
                """Project half h: q^T/k^T (packed) and v^T for local tiles
                8h..8h+7, quarter-pair interleaved per contraction chunk so
                compute tracks DMA arrival."""
                qk_ps = [psX.tile([128, 512], fp32, tag="x", name=f"qk{h}{i}") for i in range(2)]
                v_ps = [psB.tile([64, 512], fp32, tag="pv", name=f"pv{h}{i}") for i in range(2)]
                for c in range(NCH):
                    mv = xt_sb[:, h, c, :]
                    for i in range(2):
                        nc.tensor.matmul(
                            qk_ps[i][:], wqk(c), mv[:, 512 * i:512 * (i + 1)],
                            start=(c == 0), stop=(c == NCH - 1),
                        )
                    for i in range(2):
                        nc.tensor.matmul(
                            v_ps[i][:], wv(c), mv[:, 512 * i:512 * (i + 1)],
                            start=(c == 0), stop=(c == NCH - 1),
                        )
                for i in range(2):
                    sl = slice(1024 * h + 512 * i, 1024 * h + 512 * (i + 1))
                    if h == 0:
                        nc.vector.tensor_copy(out=qT_sb[:, 512 * i:512 * (i + 1)], in_=qk_ps[i][0:64, :])
                    nc.vector.tensor_copy(out=kT_sb[:, sl], in_=qk_ps[i][64:128, :])
                    nc.vector.tensor_copy(out=vT_sb[:, sl], in_=v_ps[i][:])

            def emit_vtrans(tq):
                """PE-transpose v^T tiles 4*tq..4*tq+3 into vp (v-natural)."""
                vt_ps = psX.tile([128, 8, 64], bf16, tag="x", name=f"vt{tq}")
                for i in range(4):
                    t = 4 * tq + i
                    nc.tensor.transpose(
                        vt_ps[:, i, :], vT_sb[:, 128 * t:128 * (t + 1)], idn_sb
                    )
                for i in range(4):
                    t = 4 * tq + i
                    nc.vector.tensor_copy(out=vp_sb[:, t, 0:64], in_=vt_ps[:, i, :])

            o_ps = psO.tile([H + 1, TQ], fp32, tag="o")

            def emit_S(p):
                """S^T matmul + exp per piece; boundary mask into a side
                tile on GPSIMD (off the AV critical path)."""
                a0 = 128 * (p % 8)
                pieces = [(a0, 512), (512, 1024)] if a0 < 512 else [(a0, 1024)]
                mask = tri_sb if p < 8 else gm_sb
                out = []
                for (lo, hi) in pieces:
                    w = hi - lo
                    s_ps = psX.tile([128, 512], fp32, tag="x", name=f"s{p}_{lo}")
                    nc.tensor.matmul(
                        s_ps[:, 0:w],
                        kT_sb[:, 128 * p:128 * (p + 1)],
                        qT_sb[:, lo:hi],
                        start=True, stop=True,
                    )
                    pt = ptp.tile([128, 512], bf16, tag="pt", name=f"pt{p}_{lo}")
                    nc.scalar.activation(pt[:, 0:w], s_ps[:, 0:w], Exp, scale=SCALE)
                    if lo == a0:
                        nc.gpsimd.tensor_mul(pt[:, 0:128], pt[:, 0:128], mask)
                    out.append((lo, hi, pt, 0))
                return out

            def emit_AV(p, avs):
                for (lo, hi, src_t, off) in avs:
                    nc.tensor.matmul(
                        o_ps[:, lo:hi],
                        vp_sb[:, p, 0:65],
                        src_t[:, off:off + (hi - lo)],
                        start=(p == 0), stop=(p == NKT - 1),
                        skip_group_check=True,
                    )

            def emit_half_attn(h):
                """Software pipeline: S(p) runs one tile ahead of AV(p-1) so
                the PE never waits on exp/mask latency."""
                pend = None
                for p in range(8 * h, 8 * h + 8):
                    avs = emit_S(p)
                    if pend is not None:
                        emit_AV(p - 1, pend)
                    pend = avs
                    if p == 8 * h + 4 and h == 1:
                        # cols 0:512 fully accumulated after AV(11)
                        nc.vector.tensor_copy(out=o_sb[:, 0:512], in_=o_ps[:, 0:512])
                        nc.sync.dma_start(out=out_d[:, 0:512], in_=o_sb[:, 0:512])
                emit_AV(8 * h + 7, pend)

            # own half: project, transpose v, attend. The other half of x^T
            # is still in flight during own attention.
            emit_half_proj(0)
            emit_vtrans(0)
            emit_vtrans(1)
            emit_half_attn(0)

            # other half
            emit_half_proj(1)
            emit_vtrans(2)
            emit_vtrans(3)
            emit_half_attn(1)
            nc.vector.tensor_copy(out=o_sb[:, 512:1024], in_=o_ps[:, 512:1024])
            nc.sync.dma_start(out=out_d[:, 512:1024], in_=o_sb[:, 512:1024])

    nc.finalize()
    return nc


def _get_program():
    if "nc" not in _prog_cache:
        _prog_cache["nc"] = _build_program()
    return _prog_cache["nc"]


def make_in_maps(x, Wq, Wk, Wv):
    bf16 = ml_dtypes.bfloat16
    wqk = np.concatenate([Wq, Wk], axis=1)          # [C, 128]
    wqk_p = wqk.reshape(8, 128, 128).transpose(1, 0, 2).reshape(128, 1024)
    wv_p = np.asarray(Wv).reshape(8, 128, 64).transpose(1, 0, 2).reshape(128, 512)
    cstA = np.ascontiguousarray(np.concatenate([wqk_p, wv_p], axis=1)).astype(bf16)
    tri = np.triu(np.ones((128, 128), np.float32))  # tri[k,q]=1 iff q>=k
    idn = np.zeros((128, 64), np.float32)
    idn[:64] = np.eye(64, dtype=np.float32)
    in_maps = []
    for core in range(8):
        b, r = core // 2, core % 2
        own = [2 * m + r for m in range(8)]
        other = [2 * m + 1 - r for m in range(8)]
        idx = np.concatenate([np.arange(g * 128, (g + 1) * 128) for g in own + other])
        xp = np.asarray(x[b]).T[:, idx]             # [C, 2048] permuted
        xt = xp.reshape(8, 128, 2, 1024).transpose(1, 2, 0, 3)  # [128,2,8,1024]
        gm = np.full((128, 128), 1.0 if r == 1 else 0.0, np.float32)
        cstB = np.ascontiguousarray(
            np.concatenate([tri, idn, gm], axis=1)
        ).astype(bf16)
        in_maps.append({
            "xt": np.ascontiguousarray(xt).astype(bf16),
            "cstA": cstA,
            "cstB": cstB,
        })
    return in_maps


def postprocess(results):
    out = np.empty((B, T, H), np.float32)
    for core in range(8):
        b, r = core // 2, core % 2
        oT = results[core]["outT"]  # [65, 1024]
        vals = (oT[:H] / oT[H:H + 1]).T.reshape(8, 128, H)
        ob = out[b].reshape(16, 128, H)
        for m in range(8):
            ob[2 * m + r] = vals[m]
    return out


def kernel(x, mask, Wq, Wk, Wv, _trace=False, _tracedir=None):
    from concourse import bass_utils

    nc = _get_program()
    in_maps = make_in_maps(np.asarray(x, np.float32), np.asarray(Wq, np.float32),
                           np.asarray(Wk, np.float32), np.asarray(Wv, np.float32))
    res = bass_utils.run_bass_kernel_spmd(
        nc, in_maps, core_ids=list(range(8)),
        trace=_trace, tmpdir=_tracedir,
    )
    out = postprocess(res.results)
    if _trace:
        return out, res
    return out
